# revision 10
# baseline (speedup 1.0000x reference)
"""BitSPPF kernel for Trainium2 (8 NeuronCores, data-parallel over batch).

Pipeline per core (4 images):
  cv1 (1x1 ternary conv, bf16) -> BN+SiLU (ACT) -> 3x chained 5x5 maxpool
  (bf16, DVE) -> per-channel mean-shift + fp8e4 quantize (ACT, Identity)
  -> cv2 (1x1 ternary conv, fp8 DoubleRow @ 2x PE rate) -> BN+SiLU -> DRAM.

fp8 trick: ternary weights {-1,0,+1} are exact in fp8e4. Activations are
quantized fp8 AFTER subtracting a per-channel constant mu (the analytic
mean of each SPPF branch under x~N(0,1), computed host-side from weights
alone); W2 @ mu is folded into the cv2 bias, so the shift is algebraically
exact and only shrinks quantization error (~3x vs unshifted).
"""

import os
import sys

for _p in ("/opt/trn_rl_repo",):
    if _p not in sys.path and os.path.isdir(_p):
        sys.path.insert(0, _p)

import numpy as np
import ml_dtypes

import concourse.bass as bass
import concourse.tile as tile
from concourse import bacc, mybir

BF16 = mybir.dt.bfloat16
FP8 = mybir.dt.float8e4
F32 = mybir.dt.float32
NPBF16 = ml_dtypes.bfloat16
NPFP8 = ml_dtypes.float8_e4m3

# Problem shapes (hardcoded per spec)
B, C1, H, W = 32, 1024, 40, 40
HID, C2 = 512, 1024
S = H * W  # 1600
N_CORES = 8
BL = B // N_CORES  # images per core

NEG = -3.0e38  # effectively -inf for maxpool padding, finite in bf16

EPS = 1e-8
BN_EPS = 1e-5


def _pools_chain(nc, P, HX, M2, Pout, padded_out):
    """One 5x5 stride-1 pad-2 maxpool: P -> Pout.

    P: [128, 40, 44] bf16, data in cols 2..41, cols {0,1,42,43} = NEG.
    HX: [128, 44, 40] scratch; rows {0,1,42,43} pre-set to NEG.
    M2: [128, 44, 44] scratch.
    Pout: [128, 40, 44] (padded_out=True, data to cols 2..41)
          or [128, 40, 40] (padded_out=False).
    """
    nc.vector.tensor_max(M2[:, 0:40, 0:43], P[:, :, 0:43], P[:, :, 1:44])
    nc.vector.tensor_max(HX[:, 2:42, :], M2[:, 0:40, 0:40], M2[:, 0:40, 2:42])
    nc.vector.tensor_max(HX[:, 2:42, :], HX[:, 2:42, :], P[:, :, 4:44])
    nc.vector.tensor_max(M2[:, 0:43, 0:40], HX[:, 0:43, :], HX[:, 1:44, :])
    if padded_out:
        ov = Pout[:, :, 2:42]
    else:
        ov = Pout[:, :, :]
    nc.vector.tensor_max(ov, M2[:, 0:40, 0:40], M2[:, 2:42, 0:40])
    nc.vector.tensor_max(ov, ov, HX[:, 4:44, :])


def _build_nc(bl=BL):
    nc = bacc.Bacc(trn_type="TRN2", debug=False)

    xq_d = nc.dram_tensor("xq", [bl, C1, S], BF16, kind="ExternalInput")
    w1t_d = nc.dram_tensor("w1t", [C1, HID], BF16, kind="ExternalInput")
    w2t_d = nc.dram_tensor("w2t", [4 * HID, C2], FP8, kind="ExternalInput")
    sc1_d = nc.dram_tensor("sc1", [HID], F32, kind="ExternalInput")
    bi1_d = nc.dram_tensor("bi1", [HID], F32, kind="ExternalInput")
    sc2_d = nc.dram_tensor("sc2", [C2], F32, kind="ExternalInput")
    bi2_d = nc.dram_tensor("bi2", [C2], F32, kind="ExternalInput")
    nmu_d = nc.dram_tensor("nmu", [4 * HID], F32, kind="ExternalInput")
    out_d = nc.dram_tensor("out", [bl, C2, S], F32, kind="ExternalOutput")

    KT1 = C1 // 128       # 8 k-tiles for cv1
    MT1 = HID // 128      # 4 m-tiles (= pool channel tiles)
    KP2 = 4 * HID // 256  # 8 fp8 DoubleRow k-pairs for cv2
    MT2 = C2 // 128       # 8 m-tiles for cv2
    NQ = 4                # spatial quarters of 400 cols (10 rows of 40)
    QW = S // NQ          # 400

    xv = xq_d.ap().rearrange("b (kt p) s -> b p kt s", p=128)
    ov = out_d.ap().rearrange("b (mt p) s -> b p mt s", p=128)

    # CoreSim doesn't implement Silu; allow substituting Sigmoid for
    # wiring-validation sim runs (numerics then differ by design).
    if os.environ.get("BITSPPF_SIM_ACT") == "sigmoid":
        silu = mybir.ActivationFunctionType.Sigmoid
    else:
        silu = mybir.ActivationFunctionType.Silu
    ident = mybir.ActivationFunctionType.Identity

    with tile.TileContext(nc) as tc:
        with (
            tc.tile_pool(name="const", bufs=1) as const,
            tc.tile_pool(name="xin", bufs=2) as xin,
            tc.tile_pool(name="pbuf0", bufs=2 * MT1) as pbuf0,
            tc.tile_pool(name="pbuf", bufs=MT1) as pbuf,
            tc.tile_pool(name="fbuf", bufs=2) as fbuf,
            tc.tile_pool(name="work", bufs=1) as work,
            tc.tile_pool(name="osb", bufs=2) as osb,
            tc.tile_pool(name="ps1", bufs=2, space="PSUM") as ps1p,
            tc.tile_pool(name="ps2", bufs=2, space="PSUM") as ps2p,
        ):
            # Pre-warm the ACT engine's Silu spline tables (~2.7us load)
            # during the initial DMA window.
            warm = const.tile([128, 2], F32)
            nc.vector.memset(warm, 0.0)
            nc.scalar.activation(out=warm, in_=warm, func=silu)

            w1_sb = const.tile([128, KT1, HID], BF16)
            nc.sync.dma_start(w1_sb, w1t_d.ap().rearrange("(kt p) m -> p kt m", p=128))
            sc1_sb = const.tile([128, MT1], F32)
            nc.sync.dma_start(sc1_sb, sc1_d.ap().rearrange("(t p) -> p t", p=128))
            bi1_sb = const.tile([128, MT1], F32)
            nc.sync.dma_start(bi1_sb, bi1_d.ap().rearrange("(t p) -> p t", p=128))
            nmu_sb = const.tile([128, 4 * MT1], F32)
            nc.sync.dma_start(nmu_sb, nmu_d.ap().rearrange("(t p) -> p t", p=128))

            def load_cv2_consts():
                w2_sb = const.tile([128, 2 * KP2, C2], FP8)
                nc.sync.dma_start(
                    w2_sb, w2t_d.ap().rearrange("(kt p) m -> p kt m", p=128)
                )
                sc2_sb = const.tile([128, MT2], F32)
                nc.sync.dma_start(sc2_sb, sc2_d.ap().rearrange("(t p) -> p t", p=128))
                bi2_sb = const.tile([128, MT2], F32)
                nc.sync.dma_start(bi2_sb, bi2_d.ap().rearrange("(t p) -> p t", p=128))
                return w2_sb, sc2_sb, bi2_sb

            # PE HAM warm-up: keep the PE activity window busy from the
            # moment the sc1 constants arrive until the first real matmul.
            wps = ps1p.tile([128, 2, 512], F32, tag="ps1")
            for _i in range(40):
                nc.tensor.matmul(
                    wps[0:4, 0, 0:4],
                    sc1_sb,
                    sc1_sb,
                    start=True,
                    stop=True,
                )
            for _i in range(10):
                nc.tensor.matmul(
                    wps[:, 0, 0:32],
                    w1_sb[:, 0, 0:128],
                    w1_sb[:, 0, 0:32],
                    start=True,
                    stop=True,
                )

            pimg = {}  # b -> [P0 list, P1, P2, P3]
            fimg = {}  # b -> {(level, pair): fp8 tile [128, 2, 40, 40]}

            def emit_cv1(b):
                """cv1 + fused BN/SiLU; writes h into padded P0 buffers."""
                P0 = []
                for ct in range(MT1):
                    p0 = pbuf0.tile([128, 40, 44], BF16, tag="P0")
                    nc.gpsimd.memset(p0[:, :, 0:2], NEG)
                    nc.gpsimd.memset(p0[:, :, 42:44], NEG)
                    P0.append(p0)
                pimg[b] = [P0, None, None, None]
                for qp in range(NQ // 2):   # quarter-pairs share one psum tile
                    xs = {}
                    for j in range(2):
                        q = 2 * qp + j
                        xs[j] = xin.tile([128, KT1, QW], BF16, tag="x",
                                         name="xs")
                        nc.sync.dma_start(xs[j], xv[b][:, :, q * QW:(q + 1) * QW])
                    for mt in range(MT1):
                        ps = ps1p.tile([128, 2, 512], F32, tag="ps1")
                        for j in range(2):
                            for kt in range(KT1):
                                nc.tensor.matmul(
                                    ps[:, j, :QW],
                                    w1_sb[:, kt, mt * 128:(mt + 1) * 128],
                                    xs[j][:, kt, :],
                                    start=(kt == 0),
                                    stop=(kt == KT1 - 1),
                                )
                        nc.scalar.activation(
                            out=P0[mt][:, 2 * qp * 10:(2 * qp + 2) * 10, 2:42],
                            in_=ps[:, :, :QW],
                            func=silu,
                            bias=bi1_sb[:, mt:mt + 1],
                            scale=sc1_sb[:, mt:mt + 1],
                        )

            def emit_conv(b, level, ct, src_view):
                """ACT: fp8 quantize with per-channel mean shift."""
                ftile = fimg[b][(level, ct // 2)]
                col = level * MT1 + ct
                nc.scalar.activation(
                    out=ftile[:, ct % 2],
                    in_=src_view,
                    func=ident,
                    bias=nmu_sb[:, col:col + 1],
                    scale=1.0,
                )

            def emit_pools(b):
                P0 = pimg[b][0]
                fimg[b] = {
                    (lv, j): fbuf.tile([128, 2, 40, 40], FP8, tag=f"F{lv}{j}",
                                       name=f"F{lv}{j}")
                    for lv in range(4) for j in range(2)
                }
                P1, P2, P3 = [], [], []
                for ct in range(MT1):
                    emit_conv(b, 0, ct, P0[ct][:, :, 2:42])
                    HX = work.tile([128, 44, 40], BF16, tag="HX")
                    M2 = work.tile([128, 44, 44], BF16, tag="M2")
                    nc.gpsimd.memset(HX[:, 0:2, :], NEG)
                    nc.gpsimd.memset(HX[:, 42:44, :], NEG)
                    p1 = pbuf.tile([128, 40, 44], BF16, tag="P1")
                    p2 = pbuf.tile([128, 40, 44], BF16, tag="P2")
                    p3 = pbuf.tile([128, 40, 40], BF16, tag="P3")
                    for pp in (p1, p2):
                        nc.gpsimd.memset(pp[:, :, 0:2], NEG)
                        nc.gpsimd.memset(pp[:, :, 42:44], NEG)
                    _pools_chain(nc, P0[ct], HX, M2, p1, True)
                    emit_conv(b, 1, ct, p1[:, :, 2:42])
                    _pools_chain(nc, p1, HX, M2, p2, True)
                    emit_conv(b, 2, ct, p2[:, :, 2:42])
                    _pools_chain(nc, p2, HX, M2, p3, False)
                    emit_conv(b, 3, ct, p3[:, :, :])
                    P1.append(p1)
                    P2.append(p2)
                    P3.append(p3)
                pimg[b][1] = P1
                pimg[b][2] = P2
                pimg[b][3] = P3

            def emit_cv2(b):
                """fp8 DoubleRow: each MM contracts a (level, ct-pair) = K 256."""
                F = fimg[b]
                for mt2 in range(MT2):
                    psA = ps2p.tile([128, 2, 512], F32, tag="ps2")
                    psB = ps2p.tile([128, 2, 512], F32, tag="ps2")
                    for kp in range(KP2):
                        lhs = w2_sb[:, 2 * kp:2 * kp + 2, mt2 * 128:(mt2 + 1) * 128]
                        ftile = F[(kp // 2, kp % 2)]
                        st = kp == 0
                        sp = kp == KP2 - 1
                        for qi, pst in ((0, psA[:, 0, :QW]), (1, psA[:, 1, :QW]),
                                        (2, psB[:, 0, :QW]), (3, psB[:, 1, :QW])):
                            nc.tensor.matmul(
                                pst, lhs,
                                ftile[:, :, qi * 10:(qi + 1) * 10, :],
                                start=st, stop=sp,
                                perf_mode=mybir.MatmulPerfMode.DoubleRow,
                            )
                    oa = osb.tile([128, 800], F32, tag="o")
                    nc.scalar.activation(
                        out=oa, in_=psA[:, :, :QW], func=silu,
                        bias=bi2_sb[:, mt2:mt2 + 1], scale=sc2_sb[:, mt2:mt2 + 1],
                    )
                    nc.sync.dma_start(ov[b][:, mt2, 0:800], oa)
                    ob = osb.tile([128, 800], F32, tag="o")
                    nc.scalar.activation(
                        out=ob, in_=psB[:, :, :QW], func=silu,
                        bias=bi2_sb[:, mt2:mt2 + 1], scale=sc2_sb[:, mt2:mt2 + 1],
                    )
                    nc.sync.dma_start(ov[b][:, mt2, 800:1600], ob)

            # Software pipeline: cv2(b) is emitted `lag` images behind cv1(b)
            # so the PE always has matmul work while pools+quantize complete
            # on DVE/ACT.
            lag = 1
            w2_refs = None
            for b in range(bl):
                emit_cv1(b)
                if b == 0:
                    w2_refs = load_cv2_consts()
                    w2_sb, sc2_sb, bi2_sb = w2_refs
                if b >= lag:
                    emit_cv2(b - lag)
                emit_pools(b)
            for b in range(max(0, bl - lag), bl):
                emit_cv2(b)

    nc.compile()
    return nc


_NC_CACHE = {}


def _get_nc(bl=BL):
    if bl not in _NC_CACHE:
        _NC_CACHE[bl] = _build_nc(bl)
    return _NC_CACHE[bl]


def _analytic_mu(t1, sc1, bi1):
    """mu[level, c] = E[max_m h_c], m in {1,25,81,169}, h_c = silu(a_c),
    a_c ~ N(bi1_c, sc1_c^2 * nnz_c) -- from x ~ N(0,1) iid (spec fill)."""
    C = t1.shape[0]
    nnz = np.sum(t1 != 0, axis=1).astype(np.float64)
    z = np.linspace(-8.5, 8.5, 4001)
    dz = z[1] - z[0]
    wts = np.exp(-0.5 * z * z) / np.sqrt(2 * np.pi) * dz
    a = bi1[:, None].astype(np.float64) \
        + (sc1.astype(np.float64) * np.sqrt(nnz))[:, None] * z[None, :]
    hgrid = a / (1.0 + np.exp(-a))  # [C, 4001]
    order = np.argsort(hgrid, axis=1)
    hs = np.take_along_axis(hgrid, order, axis=1)
    ws = np.broadcast_to(wts, hgrid.shape)
    ws = np.take_along_axis(ws, order, axis=1)
    cdf = np.cumsum(ws, axis=1)
    cdf = np.clip(cdf / cdf[:, -1:], 0.0, 1.0)
    mus = np.zeros((4, C))
    mus[0] = np.sum(hs * ws, axis=1)
    for li, m in enumerate([25, 81, 169]):
        Fm = cdf ** m
        pmf = np.diff(np.concatenate([np.zeros((C, 1)), Fm], axis=1), axis=1)
        mus[li + 1] = np.sum(hs * pmf, axis=1)
    return mus.astype(np.float32)


def _prep(inputs):
    """Host-side: quantize weights to ternary, fold BitNet scale + BN into
    per-channel (scale, bias), compute analytic branch means mu and fold
    W2 @ mu into the cv2 bias."""
    x = np.asarray(inputs["x"], dtype=np.float32)
    w1 = np.asarray(inputs["w1"], dtype=np.float32)
    w2 = np.asarray(inputs["w2"], dtype=np.float32)
    g1 = np.asarray(inputs["g1"], dtype=np.float32)
    b1 = np.asarray(inputs["b1"], dtype=np.float32)
    m1 = np.asarray(inputs["m1"], dtype=np.float32)
    v1 = np.asarray(inputs["v1"], dtype=np.float32)
    g2 = np.asarray(inputs["g2"], dtype=np.float32)
    b2 = np.asarray(inputs["b2"], dtype=np.float32)
    m2 = np.asarray(inputs["m2"], dtype=np.float32)
    v2 = np.asarray(inputs["v2"], dtype=np.float32)

    def fold(w, g, b, m, v):
        s = np.float32(max(np.median(np.abs(w)), EPS))
        t = np.clip(np.round(w / s), -1.0, 1.0).astype(np.float32)
        inv = g / np.sqrt(v + BN_EPS)
        scale = (s * inv).astype(np.float32)
        bias = (b - m * inv).astype(np.float32)
        return t, scale, bias

    t1, sc1, bi1 = fold(w1, g1, b1, m1, v1)
    t2, sc2, bi2 = fold(w2, g2, b2, m2, v2)

    mu = _analytic_mu(t1, sc1, bi1)          # [4, HID]
    # exact algebraic correction: W2 @ cat = W2 @ (cat - mu) + W2 @ mu
    bi2 = (bi2 + sc2 * (t2 @ mu.reshape(-1))).astype(np.float32)
    # flat order (level, ct, p) == level-major channel order == "(t p) -> p t"
    nmu = (-mu.reshape(-1)).astype(np.float32)

    w1t = np.ascontiguousarray(t1.T).astype(NPBF16)
    w2t = np.ascontiguousarray(t2.T).astype(NPFP8)

    xq = x.reshape(B, C1, S).astype(NPBF16)
    shared = dict(w1t=w1t, w2t=w2t, sc1=sc1, bi1=bi1, sc2=sc2, bi2=bi2,
                  nmu=nmu)
    in_maps = []
    for d in range(N_CORES):
        m = dict(shared)
        m["xq"] = np.ascontiguousarray(xq[d * BL:(d + 1) * BL])
        in_maps.append(m)
    return in_maps


def _install_ntff_hook():
    """The agent image's antenv lacks axon_hooks; synthesize it so
    run_bass_kernel_spmd(trace=True) can capture NTFF profiles."""
    import types

    try:
        import antenv.axon_hooks  # noqa: F401

        return
    except ImportError:
        pass
    try:
        import antenv

        bootdir = "/root/.axon_site/trn_agent_boot"
        if bootdir not in sys.path and os.path.isdir(bootdir):
            sys.path.insert(0, bootdir)
        import trn_boot

        hook = trn_boot._ntff_profile_via_ctypes("/opt/axon/libaxon_pjrt.so")
        mod = types.ModuleType("antenv.axon_hooks")
        state = {"h": hook}
        mod.get_axon_ntff_profile_hook = lambda: state["h"]
        mod.set_axon_ntff_profile_hook = lambda h: state.update(h=h)
        sys.modules["antenv.axon_hooks"] = mod
        antenv.axon_hooks = mod
    except Exception as e:  # profiling is best-effort; execution still works
        print(f"ntff hook install failed: {e}", file=sys.stderr)


def _run(inputs, trace=False):
    from concourse import bass_utils

    if trace:
        _install_ntff_hook()
    nc = _get_nc()
    in_maps = _prep(inputs)
    import time

    res = None
    for attempt, delay in ((0, 5), (1, 20), (2, 0)):
        try:
            res = bass_utils.run_bass_kernel_spmd(
                nc, in_maps, core_ids=list(range(N_CORES)), trace=trace,
            )
            break
        except Exception as e:  # transient device errors happen; back off
            if attempt == 2:
                raise
            print(
                f"run_bass_kernel_spmd failed ({type(e).__name__}); "
                f"retrying in {delay}s",
                file=sys.stderr,
            )
            time.sleep(delay)
    assert res is not None
    outs = [res.results[d]["out"] for d in range(N_CORES)]
    full = np.concatenate(outs, axis=0).reshape(B, C2, H, W).astype(np.float32)
    return full, res


def kernel(**inputs):
    full, _ = _run(inputs, trace=False)
    return full


def run_traced(**inputs):
    full, res = _run(inputs, trace=True)
    return full, res.exec_time_ns


# revision 16
# speedup vs baseline: 1.0856x; 1.0856x over previous
"""BitSPPF kernel for Trainium2 (8 NeuronCores, data-parallel over batch).

Pipeline per core (4 images):
  cv1 (1x1 ternary conv, bf16) -> BN+SiLU (ACT) -> 3x chained 5x5 maxpool
  (bf16, DVE) -> per-channel mean-shift + fp8e4 quantize (ACT, Identity)
  -> cv2 (1x1 ternary conv, fp8 DoubleRow @ 2x PE rate) -> BN+SiLU -> DRAM.

fp8 trick: ternary weights {-1,0,+1} are exact in fp8e4. Activations are
quantized fp8 AFTER subtracting a per-channel constant mu (the analytic
mean of each SPPF branch under x~N(0,1), computed host-side from weights
alone); W2 @ mu is folded into the cv2 bias, so the shift is algebraically
exact and only shrinks quantization error (~3x vs unshifted).
"""

import os
import sys

for _p in ("/opt/trn_rl_repo",):
    if _p not in sys.path and os.path.isdir(_p):
        sys.path.insert(0, _p)

import numpy as np
import ml_dtypes

import concourse.bass as bass
import concourse.tile as tile
from concourse import bacc, mybir

BF16 = mybir.dt.bfloat16
FP8 = mybir.dt.float8e4
F32 = mybir.dt.float32
NPBF16 = ml_dtypes.bfloat16
NPFP8 = ml_dtypes.float8_e4m3

# Problem shapes (hardcoded per spec)
B, C1, H, W = 32, 1024, 40, 40
HID, C2 = 512, 1024
S = H * W  # 1600
N_CORES = 8
BL = B // N_CORES  # images per core

NEG = -3.0e38  # effectively -inf for maxpool padding, finite in bf16

EPS = 1e-8
BN_EPS = 1e-5


def _pools_chain(nc, P, HX, M2, Pout, padded_out):
    """One 5x5 stride-1 pad-2 maxpool: P -> Pout.

    P: [128, 40, 44] bf16, data in cols 2..41, cols {0,1,42,43} = NEG.
    HX: [128, 44, 40] scratch; rows {0,1,42,43} pre-set to NEG.
    M2: [128, 44, 44] scratch.
    Pout: [128, 40, 44] (padded_out=True, data to cols 2..41)
          or [128, 40, 40] (padded_out=False).
    """
    nc.vector.tensor_max(M2[:, 0:40, 0:43], P[:, :, 0:43], P[:, :, 1:44])
    nc.vector.tensor_max(HX[:, 2:42, :], M2[:, 0:40, 0:40], M2[:, 0:40, 2:42])
    nc.vector.tensor_max(HX[:, 2:42, :], HX[:, 2:42, :], P[:, :, 4:44])
    nc.vector.tensor_max(M2[:, 0:43, 0:40], HX[:, 0:43, :], HX[:, 1:44, :])
    if padded_out:
        ov = Pout[:, :, 2:42]
    else:
        ov = Pout[:, :, :]
    nc.vector.tensor_max(ov, M2[:, 0:40, 0:40], M2[:, 2:42, 0:40])
    nc.vector.tensor_max(ov, ov, HX[:, 4:44, :])


def _build_nc(bl=BL):
    nc = bacc.Bacc(trn_type="TRN2", debug=False)

    xq_d = nc.dram_tensor("xq", [bl, C1, S], BF16, kind="ExternalInput")
    w1t_d = nc.dram_tensor("w1t", [C1, HID], BF16, kind="ExternalInput")
    w2t_d = nc.dram_tensor("w2t", [4 * HID, C2], FP8, kind="ExternalInput")
    sc1_d = nc.dram_tensor("sc1", [HID], F32, kind="ExternalInput")
    bi1_d = nc.dram_tensor("bi1", [HID], F32, kind="ExternalInput")
    sc2_d = nc.dram_tensor("sc2", [C2], F32, kind="ExternalInput")
    bi2_d = nc.dram_tensor("bi2", [C2], F32, kind="ExternalInput")
    nmu_d = nc.dram_tensor("nmu", [4 * HID], F32, kind="ExternalInput")
    out_d = nc.dram_tensor("out", [bl, C2, S], F32, kind="ExternalOutput")

    KT1 = C1 // 128       # 8 k-tiles for cv1
    MT1 = HID // 128      # 4 m-tiles (= pool channel tiles)
    KP2 = 4 * HID // 256  # 8 fp8 DoubleRow k-pairs for cv2
    MT2 = C2 // 128       # 8 m-tiles for cv2
    NQ = 4                # spatial quarters of 400 cols (10 rows of 40)
    QW = S // NQ          # 400

    xv = xq_d.ap().rearrange("b (kt p) s -> b p kt s", p=128)
    ov = out_d.ap().rearrange("b (mt p) s -> b p mt s", p=128)

    # CoreSim doesn't implement Silu; allow substituting Sigmoid for
    # wiring-validation sim runs (numerics then differ by design).
    if os.environ.get("BITSPPF_SIM_ACT") == "sigmoid":
        silu = mybir.ActivationFunctionType.Sigmoid
    else:
        silu = mybir.ActivationFunctionType.Silu
    ident = mybir.ActivationFunctionType.Identity

    with tile.TileContext(nc) as tc:
        with (
            tc.tile_pool(name="const", bufs=1) as const,
            tc.tile_pool(name="xin", bufs=3) as xin,
            tc.tile_pool(name="pbuf0", bufs=2 * MT1) as pbuf0,
            tc.tile_pool(name="pbuf", bufs=MT1) as pbuf,
            tc.tile_pool(name="fbuf", bufs=3) as fbuf,
            tc.tile_pool(name="work", bufs=1) as work,
            tc.tile_pool(name="osb", bufs=2) as osb,
            tc.tile_pool(name="ps1", bufs=2, space="PSUM") as ps1p,
            tc.tile_pool(name="ps2", bufs=3, space="PSUM") as ps2p,
        ):
            # Pre-warm the ACT engine's Silu spline tables (~2.7us load)
            # during the initial DMA window.
            warm = const.tile([128, 2], F32)
            nc.vector.memset(warm, 0.0)
            nc.scalar.activation(out=warm, in_=warm, func=silu)

            w1_sb = const.tile([128, KT1, HID], BF16)
            nc.sync.dma_start(w1_sb, w1t_d.ap().rearrange("(kt p) m -> p kt m", p=128))
            sc1_sb = const.tile([128, MT1], F32)
            nc.sync.dma_start(sc1_sb, sc1_d.ap().rearrange("(t p) -> p t", p=128))
            bi1_sb = const.tile([128, MT1], F32)
            nc.sync.dma_start(bi1_sb, bi1_d.ap().rearrange("(t p) -> p t", p=128))
            nmu_sb = const.tile([128, 4 * MT1], F32)
            nc.sync.dma_start(nmu_sb, nmu_d.ap().rearrange("(t p) -> p t", p=128))

            def load_cv2_consts():
                w2_sb = const.tile([128, 2 * KP2, C2], FP8)
                nc.sync.dma_start(
                    w2_sb, w2t_d.ap().rearrange("(kt p) m -> p kt m", p=128)
                )
                sc2_sb = const.tile([128, MT2], F32)
                nc.sync.dma_start(sc2_sb, sc2_d.ap().rearrange("(t p) -> p t", p=128))
                bi2_sb = const.tile([128, MT2], F32)
                nc.sync.dma_start(bi2_sb, bi2_d.ap().rearrange("(t p) -> p t", p=128))
                return w2_sb, sc2_sb, bi2_sb

            # PE HAM warm-up: keep the PE activity window busy from the
            # moment the sc1 constants arrive until the first real matmul.
            wps = ps1p.tile([128, 512], F32, tag="ps1")
            for _i in range(40):
                nc.tensor.matmul(
                    wps[0:4, 0:4],
                    sc1_sb,
                    sc1_sb,
                    start=True,
                    stop=True,
                )
            for _i in range(10):
                nc.tensor.matmul(
                    wps[:, 0:32],
                    w1_sb[:, 0, 0:128],
                    w1_sb[:, 0, 0:32],
                    start=True,
                    stop=True,
                )

            pimg = {}  # b -> [P0 list, P1, P2, P3]
            fimg = {}  # b -> {(level, pair): fp8 tile [128, 2, 40, 40]}

            def emit_cv1(b):
                """cv1 + fused BN/SiLU; writes h into padded P0 buffers."""
                P0 = []
                for ct in range(MT1):
                    p0 = pbuf0.tile([128, 40, 44], BF16, tag="P0")
                    nc.gpsimd.memset(p0[:, :, 0:2], NEG)
                    nc.gpsimd.memset(p0[:, :, 42:44], NEG)
                    P0.append(p0)
                pimg[b] = [P0, None, None, None]
                for q in range(NQ):
                    xs = xin.tile([128, KT1, QW], BF16, tag="x")
                    nc.sync.dma_start(xs, xv[b][:, :, q * QW:(q + 1) * QW])
                    for mt in range(MT1):
                        ps = ps1p.tile([128, 512], F32, tag="ps1")
                        for kt in range(KT1):
                            nc.tensor.matmul(
                                ps[:, :QW],
                                w1_sb[:, kt, mt * 128:(mt + 1) * 128],
                                xs[:, kt, :],
                                start=(kt == 0),
                                stop=(kt == KT1 - 1),
                            )
                        nc.scalar.activation(
                            out=P0[mt][:, q * 10:(q + 1) * 10, 2:42],
                            in_=ps[:, :QW],
                            func=silu,
                            bias=bi1_sb[:, mt:mt + 1],
                            scale=sc1_sb[:, mt:mt + 1],
                        )

            def emit_conv(b, level, ct, src_view):
                """ACT: fp8 quantize with per-channel mean shift."""
                ftile = fimg[b][(level, ct // 2)]
                col = level * MT1 + ct
                nc.scalar.activation(
                    out=ftile[:, ct % 2],
                    in_=src_view,
                    func=ident,
                    bias=nmu_sb[:, col:col + 1],
                    scale=1.0,
                )

            def emit_pools(b):
                P0 = pimg[b][0]
                fimg[b] = {
                    (lv, j): fbuf.tile([128, 2, 40, 40], FP8, tag=f"F{lv}{j}",
                                       name=f"F{lv}{j}")
                    for lv in range(4) for j in range(2)
                }
                P1, P2, P3 = [], [], []
                for ct in range(MT1):
                    emit_conv(b, 0, ct, P0[ct][:, :, 2:42])
                    HX = work.tile([128, 44, 40], BF16, tag="HX")
                    M2 = work.tile([128, 44, 44], BF16, tag="M2")
                    nc.gpsimd.memset(HX[:, 0:2, :], NEG)
                    nc.gpsimd.memset(HX[:, 42:44, :], NEG)
                    p1 = pbuf.tile([128, 40, 44], BF16, tag="P1")
                    p2 = pbuf.tile([128, 40, 44], BF16, tag="P2")
                    p3 = pbuf.tile([128, 40, 40], BF16, tag="P3")
                    for pp in (p1, p2):
                        nc.gpsimd.memset(pp[:, :, 0:2], NEG)
                        nc.gpsimd.memset(pp[:, :, 42:44], NEG)
                    _pools_chain(nc, P0[ct], HX, M2, p1, True)
                    emit_conv(b, 1, ct, p1[:, :, 2:42])
                    _pools_chain(nc, p1, HX, M2, p2, True)
                    emit_conv(b, 2, ct, p2[:, :, 2:42])
                    _pools_chain(nc, p2, HX, M2, p3, False)
                    emit_conv(b, 3, ct, p3[:, :, :])
                    P1.append(p1)
                    P2.append(p2)
                    P3.append(p3)
                pimg[b][1] = P1
                pimg[b][2] = P2
                pimg[b][3] = P3

            def emit_cv2(b):
                """fp8 DoubleRow: each MM contracts a (level, ct-pair) = K 256."""
                F = fimg[b]
                for mt2 in range(MT2):
                    psA = ps2p.tile([128, 2, 512], F32, tag="ps2")
                    psB = ps2p.tile([128, 2, 512], F32, tag="ps2")
                    for kp in range(KP2):
                        lhs = w2_sb[:, 2 * kp:2 * kp + 2, mt2 * 128:(mt2 + 1) * 128]
                        ftile = F[(kp // 2, kp % 2)]
                        st = kp == 0
                        sp = kp == KP2 - 1
                        for qi, pst in ((0, psA[:, 0, :QW]), (1, psA[:, 1, :QW]),
                                        (2, psB[:, 0, :QW]), (3, psB[:, 1, :QW])):
                            nc.tensor.matmul(
                                pst, lhs,
                                ftile[:, :, qi * 10:(qi + 1) * 10, :],
                                start=st, stop=sp,
                                perf_mode=mybir.MatmulPerfMode.DoubleRow,
                            )
                    oa = osb.tile([128, 800], F32, tag="o")
                    nc.scalar.activation(
                        out=oa, in_=psA[:, :, :QW], func=silu,
                        bias=bi2_sb[:, mt2:mt2 + 1], scale=sc2_sb[:, mt2:mt2 + 1],
                    )
                    nc.sync.dma_start(ov[b][:, mt2, 0:800], oa)
                    ob = osb.tile([128, 800], F32, tag="o")
                    nc.scalar.activation(
                        out=ob, in_=psB[:, :, :QW], func=silu,
                        bias=bi2_sb[:, mt2:mt2 + 1], scale=sc2_sb[:, mt2:mt2 + 1],
                    )
                    nc.sync.dma_start(ov[b][:, mt2, 800:1600], ob)

            # Software pipeline: cv2(b) is emitted `lag` images behind cv1(b)
            # so the PE always has matmul work while pools+quantize complete
            # on DVE/ACT. Pools (and their ACT conversions) are emitted BEFORE
            # cv2 so conversions never queue behind cv2-silus in ACT's strict
            # FIFO (that ordering chains PE->ACT->DVE stalls).
            lag = 2 if bl > 2 else 1
            w2_refs = None
            for b in range(bl):
                emit_cv1(b)
                if b == 0:
                    w2_refs = load_cv2_consts()
                    w2_sb, sc2_sb, bi2_sb = w2_refs
                emit_pools(b)
                if b >= lag:
                    emit_cv2(b - lag)
            for b in range(max(0, bl - lag), bl):
                emit_cv2(b)

    nc.compile()
    return nc


_NC_CACHE = {}


def _get_nc(bl=BL):
    if bl not in _NC_CACHE:
        _NC_CACHE[bl] = _build_nc(bl)
    return _NC_CACHE[bl]


def _analytic_mu(t1, sc1, bi1):
    """mu[level, c] = E[max_m h_c], m in {1,25,81,169}, h_c = silu(a_c),
    a_c ~ N(bi1_c, sc1_c^2 * nnz_c) -- from x ~ N(0,1) iid (spec fill)."""
    C = t1.shape[0]
    nnz = np.sum(t1 != 0, axis=1).astype(np.float64)
    z = np.linspace(-8.5, 8.5, 4001)
    dz = z[1] - z[0]
    wts = np.exp(-0.5 * z * z) / np.sqrt(2 * np.pi) * dz
    a = bi1[:, None].astype(np.float64) \
        + (sc1.astype(np.float64) * np.sqrt(nnz))[:, None] * z[None, :]
    hgrid = a / (1.0 + np.exp(-a))  # [C, 4001]
    order = np.argsort(hgrid, axis=1)
    hs = np.take_along_axis(hgrid, order, axis=1)
    ws = np.broadcast_to(wts, hgrid.shape)
    ws = np.take_along_axis(ws, order, axis=1)
    cdf = np.cumsum(ws, axis=1)
    cdf = np.clip(cdf / cdf[:, -1:], 0.0, 1.0)
    mus = np.zeros((4, C))
    mus[0] = np.sum(hs * ws, axis=1)
    for li, m in enumerate([25, 81, 169]):
        Fm = cdf ** m
        pmf = np.diff(np.concatenate([np.zeros((C, 1)), Fm], axis=1), axis=1)
        mus[li + 1] = np.sum(hs * pmf, axis=1)
    return mus.astype(np.float32)


def _prep(inputs):
    """Host-side: quantize weights to ternary, fold BitNet scale + BN into
    per-channel (scale, bias), compute analytic branch means mu and fold
    W2 @ mu into the cv2 bias."""
    x = np.asarray(inputs["x"], dtype=np.float32)
    w1 = np.asarray(inputs["w1"], dtype=np.float32)
    w2 = np.asarray(inputs["w2"], dtype=np.float32)
    g1 = np.asarray(inputs["g1"], dtype=np.float32)
    b1 = np.asarray(inputs["b1"], dtype=np.float32)
    m1 = np.asarray(inputs["m1"], dtype=np.float32)
    v1 = np.asarray(inputs["v1"], dtype=np.float32)
    g2 = np.asarray(inputs["g2"], dtype=np.float32)
    b2 = np.asarray(inputs["b2"], dtype=np.float32)
    m2 = np.asarray(inputs["m2"], dtype=np.float32)
    v2 = np.asarray(inputs["v2"], dtype=np.float32)

    def fold(w, g, b, m, v):
        s = np.float32(max(np.median(np.abs(w)), EPS))
        t = np.clip(np.round(w / s), -1.0, 1.0).astype(np.float32)
        inv = g / np.sqrt(v + BN_EPS)
        scale = (s * inv).astype(np.float32)
        bias = (b - m * inv).astype(np.float32)
        return t, scale, bias

    t1, sc1, bi1 = fold(w1, g1, b1, m1, v1)
    t2, sc2, bi2 = fold(w2, g2, b2, m2, v2)

    mu = _analytic_mu(t1, sc1, bi1)          # [4, HID]
    # exact algebraic correction: W2 @ cat = W2 @ (cat - mu) + W2 @ mu
    bi2 = (bi2 + sc2 * (t2 @ mu.reshape(-1))).astype(np.float32)
    # flat order (level, ct, p) == level-major channel order == "(t p) -> p t"
    nmu = (-mu.reshape(-1)).astype(np.float32)

    w1t = np.ascontiguousarray(t1.T).astype(NPBF16)
    w2t = np.ascontiguousarray(t2.T).astype(NPFP8)

    xq = x.reshape(B, C1, S).astype(NPBF16)
    shared = dict(w1t=w1t, w2t=w2t, sc1=sc1, bi1=bi1, sc2=sc2, bi2=bi2,
                  nmu=nmu)
    in_maps = []
    for d in range(N_CORES):
        m = dict(shared)
        m["xq"] = np.ascontiguousarray(xq[d * BL:(d + 1) * BL])
        in_maps.append(m)
    return in_maps


def _install_ntff_hook():
    """The agent image's antenv lacks axon_hooks; synthesize it so
    run_bass_kernel_spmd(trace=True) can capture NTFF profiles."""
    import types

    try:
        import antenv.axon_hooks  # noqa: F401

        return
    except ImportError:
        pass
    try:
        import antenv

        bootdir = "/root/.axon_site/trn_agent_boot"
        if bootdir not in sys.path and os.path.isdir(bootdir):
            sys.path.insert(0, bootdir)
        import trn_boot

        hook = trn_boot._ntff_profile_via_ctypes("/opt/axon/libaxon_pjrt.so")
        mod = types.ModuleType("antenv.axon_hooks")
        state = {"h": hook}
        mod.get_axon_ntff_profile_hook = lambda: state["h"]
        mod.set_axon_ntff_profile_hook = lambda h: state.update(h=h)
        sys.modules["antenv.axon_hooks"] = mod
        antenv.axon_hooks = mod
    except Exception as e:  # profiling is best-effort; execution still works
        print(f"ntff hook install failed: {e}", file=sys.stderr)


def _run(inputs, trace=False):
    from concourse import bass_utils

    if trace:
        _install_ntff_hook()
    nc = _get_nc()
    in_maps = _prep(inputs)
    import time

    res = None
    for attempt, delay in ((0, 5), (1, 20), (2, 0)):
        try:
            res = bass_utils.run_bass_kernel_spmd(
                nc, in_maps, core_ids=list(range(N_CORES)), trace=trace,
            )
            break
        except Exception as e:  # transient device errors happen; back off
            if attempt == 2:
                raise
            print(
                f"run_bass_kernel_spmd failed ({type(e).__name__}); "
                f"retrying in {delay}s",
                file=sys.stderr,
            )
            time.sleep(delay)
    assert res is not None
    outs = [res.results[d]["out"] for d in range(N_CORES)]
    full = np.concatenate(outs, axis=0).reshape(B, C2, H, W).astype(np.float32)
    return full, res


def kernel(**inputs):
    full, _ = _run(inputs, trace=False)
    return full


def run_traced(**inputs):
    full, res = _run(inputs, trace=True)
    return full, res.exec_time_ns
